# revision 2
# baseline (speedup 1.0000x reference)
"""MiniTransformerLayer on 8 Trainium2 NeuronCores — fp8 DoubleRow version.

Precision plan (from one-at-a-time e4m3 error attribution): every attention
-path tensor (h, w_qkv, roped q/k, v, probs, attn-out, w_out) tolerates plain
fp8 — softmax + averaging wash those errors to ~1e-3 each.  The MLP terms
(h2, w_fc1, gelu-out, w_fc2) each cost ~1.6-1.8e-2 alone, so all four are
carried as fp8 hi+lo pairs.  The lo part is quantized at the SAME scale as
hi (the residual lands in e4m3's normal/subnormal range, net error ~1e-3),
so hi- and lo-chains accumulate into one PSUM with no scale juggling.

Layout/sharding:
  - tokens t = b*S + s flattened; core c owns tokens [512c, 512(c+1)) and
    heads {2c, 2c+1}.  LN1 on own tokens -> h fp8 -> AllGather.
  - qkv fp8 DoubleRow (K=256/instruction); q,k head-stacked ([qe_m0;qe_m1]
    on partitions) so one RoPE DVE op covers both heads; rotated halves
    stored [64m.., 2(lo|hi), TOK] fp8 so scores double-pump d=128 at Ki=64.
  - per batch: qkv for that batch's 4 token blocks, then attention per
    (head, 512-query-block) — emitted interleaved so attention's ScalarE
    exp (the phase bottleneck) overlaps the next batch's qkv matmuls.
  - attention: scores transposed [k, q]; exp with constant -3 bias (cancels
    in normalization); fp8 probs; attn@V and the ones-vector denominator
    as fp8 DoubleRow accumulations.
  - AllToAll fp8 per head -> out_proj (fp8) -> residual fp16 -> LN2
    (h2 hi+lo) -> fc1 (3 chains) -> gelu -> ff hi+lo -> fc2 (3 chains).
  - DRAM activations p-major [128, ...]: bulk transfers are single
    large-descriptor DMAs; weight loads issued up front so they prefetch
    during LN1.
"""

import sys

sys.path.insert(0, "/opt/trn_rl_repo")

import numpy as np

import concourse.bass as bass  # noqa: F401
import concourse.bacc as bacc
import concourse.tile as tile
import concourse.mybir as mybir
from concourse import bass_utils

F8 = mybir.dt.float8e4
F16 = mybir.dt.float16
F32 = mybir.dt.float32
AF = mybir.ActivationFunctionType
DR = mybir.MatmulPerfMode.DoubleRow

NCORES = 8
B, S, HID, HEADS, D, FFN = 2, 2048, 2048, 16, 128, 4096
TOK = B * S
TPC = TOK // NCORES     # 512 tokens per core
HC = HID // 128         # 16 hidden chunks
HP = HC // 2            # 8 doublerow k-pairs
FFC = FFN // 128        # 32 ffn chunks
NH = HEADS // NCORES    # 2 heads per core
SB = S // TPC           # 4 query blocks per batch
KCN = S // 128          # 16 key chunks per batch
SCALE = 1.0 / float(np.sqrt(D))
EXP_BIAS = -3.0
EPS = 1e-5

S_H = 16.0    # h after LN1
S_W = 32.0    # qkv weights
S_Q = 16.0    # roped q/k
S_V = 16.0    # v (attn-out shares it)
S_WO = 64.0   # out_proj weights
S_H2 = 16.0   # h after LN2 (hi+lo same scale)
S_W1 = 32.0   # fc1 weights (hi+lo same scale)
S_FF = 1.0    # gelu output (hi+lo same scale)
S_W2 = 32.0   # fc2 weights (hi+lo same scale)

_CACHE = {}


def _emit(nc, single_core=False):
    xT = nc.dram_tensor("xT", [128, HC * TPC], F16, kind="ExternalInput")
    wqk = nc.dram_tensor("wqk", [128, HC * 512], F8, kind="ExternalInput")
    wv = nc.dram_tensor("wv", [128, HC * 256], F8, kind="ExternalInput")
    wo = nc.dram_tensor("wo", [128, HC * 2048], F8, kind="ExternalInput")
    wf1_h = nc.dram_tensor("wf1_h", [128, HC * 4096], F8, kind="ExternalInput")
    wf1_l = nc.dram_tensor("wf1_l", [128, HC * 4096], F8, kind="ExternalInput")
    wf2_h = nc.dram_tensor("wf2_h", [128, HC * 4096], F8, kind="ExternalInput")
    wf2_l = nc.dram_tensor("wf2_l", [128, HC * 4096], F8, kind="ExternalInput")
    g1 = nc.dram_tensor("g1", [128, HC], F32, kind="ExternalInput")
    b1 = nc.dram_tensor("b1", [128, HC], F32, kind="ExternalInput")
    g2 = nc.dram_tensor("g2", [128, HC], F32, kind="ExternalInput")
    b2 = nc.dram_tensor("b2", [128, HC], F32, kind="ExternalInput")
    ropeC = nc.dram_tensor("ropeC", [128, TOK], F16, kind="ExternalInput")
    ropeS = nc.dram_tensor("ropeS", [128, TOK], F16, kind="ExternalInput")
    outT = nc.dram_tensor("outT", [128, HC * TPC], F16, kind="ExternalOutput")

    rg = [list(range(NCORES))]
    MULT, ADD = mybir.AluOpType.mult, mybir.AluOpType.add

    with tile.TileContext(nc) as tc:
        with (
            tc.tile_pool(name="const", bufs=1) as const,
            tc.tile_pool(name="dram", bufs=1, space="DRAM") as dram,
            tc.tile_pool(name="x2p", bufs=1) as x2p,
        ):
            ones_col = const.tile([128, 1], F16, tag="onc")
            nc.vector.memset(ones_col[:], 1.0)
            ones_row = const.tile([1, 128], F16, tag="onr")
            nc.vector.memset(ones_row[:], 1.0)
            ones8 = const.tile([128, 2, 16], F8, tag="on8")
            nc.vector.memset(ones8[:], 1.0)
            eps_b = const.tile([1, 1], F32, tag="epsb")
            nc.vector.memset(eps_b[:], EPS)
            zero1_b = const.tile([1, 1], F32, tag="z1b")
            nc.vector.memset(zero1_b[:], 0.0)
            zero_b = const.tile([128, 1], F32, tag="zb")
            nc.vector.memset(zero_b[:], 0.0)
            expb_b = const.tile([128, 1], F32, tag="expb")
            nc.vector.memset(expb_b[:], EXP_BIAS)
            g1_sb = const.tile([128, HC], F32, tag="g1s")
            b1_sb = const.tile([128, HC], F32, tag="b1s")
            g2_sb = const.tile([128, HC], F32, tag="g2s")
            b2_sb = const.tile([128, HC], F32, tag="b2s")
            nc.scalar.dma_start(g1_sb[:], g1[:])
            nc.scalar.dma_start(b1_sb[:], b1[:])
            nc.scalar.dma_start(g2_sb[:], g2[:])
            nc.scalar.dma_start(b2_sb[:], b2[:])

            x2_sb = x2p.tile([128, HC, TPC], F16, tag="x2")

            ag_in = dram.tile([128, HC * TPC], F8)
            if single_core:
                ag_out = dram.tile([NCORES * 128, HC * TPC], F8)
            else:
                ag_out = nc.dram_tensor(
                    "ag_out_sh", [NCORES * 128, HC * TPC], F8,
                    addr_space="Shared").ap()
            a2a_in_m = [dram.tile([NCORES * 128, TPC], F8, name=f"a2ai{m}")
                        for m in range(NH)]
            a2a_out_m = [dram.tile([NCORES * 128, TPC], F8,
                                   name=f"a2ao{m}") for m in range(NH)]

            def layernorm(x_in, put_h, lnp, psst, psbc):
                # mean/var via ones-matmul partition sums (fp16 moving -> 1
                # cycle/row); per-token affine broadcast via K=1 fp16 matmuls.
                ps_sx = psst.tile([1, TPC], F32, tag="st")
                ps_sq = psst.tile([1, TPC], F32, tag="st")
                for j in range(HC):
                    s = x_in[:, j, :]
                    sqt = lnp.tile([128, TPC], F16, tag="sqt")
                    nc.vector.tensor_mul(sqt[:], s, s)
                    nc.tensor.matmul(ps_sx[:], ones_col[:], s,
                                     start=(j == 0), stop=(j == HC - 1))
                    nc.tensor.matmul(ps_sq[:], ones_col[:], sqt[:],
                                     start=(j == 0), stop=(j == HC - 1))
                mu = lnp.tile([1, TPC], F32, tag="mu", bufs=1)
                m2 = lnp.tile([1, TPC], F32, tag="m2", bufs=1)
                var = lnp.tile([1, TPC], F32, tag="var", bufs=1)
                lnv = lnp.tile([1, TPC], F32, tag="lnv", bufs=1)
                rstd = lnp.tile([1, TPC], F16, tag="rstd", bufs=1)
                mrs = lnp.tile([1, TPC], F16, tag="mrs", bufs=1)
                nc.vector.tensor_scalar_mul(mu[:], ps_sx[:], 1.0 / HID)
                nc.vector.tensor_scalar_mul(m2[:], ps_sq[:], 1.0 / HID)
                nc.vector.tensor_mul(var[:], mu[:], mu[:])
                nc.vector.tensor_sub(var[:], m2[:], var[:])
                nc.scalar.activation(lnv[:], var[:], AF.Ln, bias=eps_b[:])
                nc.scalar.activation(rstd[:], lnv[:], AF.Exp, bias=zero1_b[:],
                                     scale=-0.5)
                nc.vector.scalar_tensor_tensor(
                    mrs[:], mu[:], -1.0, rstd[:], MULT, MULT)
                ps_c1 = psbc.tile([128, TPC], F32, tag="bc")
                ps_c0 = psbc.tile([128, TPC], F32, tag="bc")
                nc.tensor.matmul(ps_c1[:], ones_row[:], rstd[:],
                                 start=True, stop=True)
                nc.tensor.matmul(ps_c0[:], ones_row[:], mrs[:],
                                 start=True, stop=True)
                c1_sb = lnp.tile([128, TPC], F16, tag="c1", bufs=1)
                c0_sb = lnp.tile([128, TPC], F16, tag="c0", bufs=1)
                nc.scalar.activation(c1_sb[:], ps_c1[:], AF.Copy)
                nc.scalar.activation(c0_sb[:], ps_c0[:], AF.Copy)
                for j in range(HC):
                    t1 = lnp.tile([128, TPC], F16, tag="t1")
                    t2 = lnp.tile([128, TPC], F16, tag="t2")
                    nc.vector.tensor_mul(t1[:], x_in[:, j, :], c1_sb[:])
                    nc.vector.tensor_add(t2[:], t1[:], c0_sb[:])
                    put_h(j, t2, lnp)

            # ================= Stages A-D (x + early weights resident) =======
            with (
                tc.tile_pool(name="xres", bufs=1) as xres,
                tc.tile_pool(name="wB", bufs=1) as wB,
                tc.tile_pool(name="wD", bufs=1) as wD,
            ):
                x_sb = xres.tile([128, HC, TPC], F16, tag="x")
                HT2 = HC * TPC // 2
                nc.sync.dma_start(x_sb[:, 0:HC // 2, :], xT[:, 0:HT2])
                nc.sync.dma_start(x_sb[:, HC // 2:, :], xT[:, HT2:])
                # weight prefetch (runs on DMA engines during LN1)
                wqk_sb = wB.tile([128, HC, 512], F8, tag="wqk")
                wv_sb = wB.tile([128, HC, 256], F8, tag="wv")
                rC = wB.tile([128, TOK], F16, tag="rC")
                rS = wB.tile([128, TOK], F16, tag="rS")
                wo_sb = wD.tile([128, HC, 2048], F8, tag="wo")
                nc.sync.dma_start(wqk_sb[:], wqk[:])
                nc.sync.dma_start(wv_sb[:], wv[:])
                nc.sync.dma_start(rC[:], ropeC[:])
                nc.sync.dma_start(rS[:], ropeS[:])
                nc.sync.dma_start(wo_sb[:], wo[:])

                # ---- Stage A: LN1 -> h fp8 -> AllGather ----
                with tc.tile_pool(name="hpool", bufs=1) as hpool:
                    h8 = hpool.tile([128, HC, TPC], F8, tag="h8")
                    with (
                        tc.tile_pool(name="lnA", bufs=3) as lnA,
                        tc.tile_pool(name="psstA", bufs=2, space="PSUM") as psA,
                        tc.tile_pool(name="psbcA", bufs=2, space="PSUM") as pbA,
                    ):
                        def put_h1(j, t2, lnp):
                            nc.gpsimd.tensor_scalar(
                                h8[:, j, :], t2[:], g1_sb[:, j:j + 1],
                                b1_sb[:, j:j + 1], MULT, ADD)

                        layernorm(x_sb, put_h1, lnA, psA, pbA)
                    nc.sync.dma_start(ag_in[:], h8[:])

                if single_core:
                    for r in range(NCORES):   # ~12us AllGather stand-in
                        nc.sync.dma_start(
                            ag_out[r * 128:(r + 1) * 128, 0:HC * TPC // 2],
                            ag_in[:, 0:HC * TPC // 2])
                else:
                    nc.gpsimd.collective_compute(
                        "AllGather", mybir.AluOpType.bypass, replica_groups=rg,
                        ins=[ag_in.opt()], outs=[ag_out],
                    )

                # ---- Stages B+C interleaved per batch: qkv, then attention --
                QK_DQ = S_Q / (S_H * S_W)
                V_DQ = S_V / (S_H * S_W)
                EXP_SCALE = SCALE / (S_Q * S_Q)
                with tc.tile_pool(name="qkvout", bufs=1) as qkvout:
                    # q/k rotated: [64*m + d/2, lo|hi, token], head-stacked
                    qrot = qkvout.tile([128, 2, TOK], F8, tag="qrot")
                    krot = qkvout.tile([128, 2, TOK], F8, tag="krot")
                    # v: [key_in_chunk, token_chunk, m*128 + d]
                    v_sb = qkvout.tile([128, TOK // 128, NH * 128], F8,
                                       tag="v")

                    with (
                        tc.tile_pool(name="htbp", bufs=2) as htbp,
                        tc.tile_pool(name="qkpre", bufs=2) as qkpre,
                        tc.tile_pool(name="ropet", bufs=2) as ropet,
                        tc.tile_pool(name="psq", bufs=1, space="PSUM") as psq,
                        tc.tile_pool(name="psv", bufs=2, space="PSUM") as psv,
                    ):
                        def qkv_tb(tb):
                            ht = htbp.tile([128, HC, TPC], F8, tag="ht")
                            nc.sync.dma_start(
                                ht[:], ag_out[tb * 128:(tb + 1) * 128, :])
                            ts_ = slice(tb * TPC, (tb + 1) * TPC)
                            # strips: 0=q_even 1=q_odd 2=k_even 3=k_odd
                            pres = []
                            for s4 in range(4):
                                ps = psq.tile([128, TPC], F32, tag=f"qk{s4}")
                                for jp in range(HP):
                                    nc.tensor.matmul(
                                        ps[:],
                                        wqk_sb[:, 2 * jp:2 * jp + 2,
                                               s4 * 128:(s4 + 1) * 128],
                                        ht[:, 2 * jp:2 * jp + 2, :],
                                        start=(jp == 0), stop=(jp == HP - 1),
                                        perf_mode=DR,
                                    )
                                pre = qkpre.tile([128, TPC], F16,
                                                 tag=f"pre{s4}")
                                nc.scalar.activation(pre[:], ps[:], AF.Copy,
                                                     scale=QK_DQ)
                                pres.append(pre)
                            for (pe_, po_, rbuf) in ((pres[0], pres[1], qrot),
                                                     (pres[2], pres[3], krot)):
                                t1 = ropet.tile([128, TPC], F16, tag="t1")
                                t2 = ropet.tile([128, TPC], F16, tag="t2")
                                t3 = ropet.tile([128, TPC], F16, tag="t3")
                                t4 = ropet.tile([128, TPC], F16, tag="t4")
                                nc.vector.tensor_mul(t1[:], pe_[:], rC[:, ts_])
                                nc.vector.tensor_mul(t2[:], po_[:], rS[:, ts_])
                                nc.vector.tensor_sub(rbuf[:, 0, ts_],
                                                     t1[:], t2[:])
                                nc.vector.tensor_mul(t3[:], pe_[:], rS[:, ts_])
                                nc.vector.tensor_mul(t4[:], po_[:], rC[:, ts_])
                                nc.vector.tensor_add(rbuf[:, 1, ts_],
                                                     t3[:], t4[:])
                            for mt in range(4):
                                psvt = psv.tile([128, NH * 128], F32, tag="v")
                                msl = slice(mt * 128, (mt + 1) * 128)
                                for jp in range(HP):
                                    nc.tensor.matmul(
                                        psvt[:],
                                        ht[:, 2 * jp:2 * jp + 2, msl],
                                        wv_sb[:, 2 * jp:2 * jp + 2, :],
                                        start=(jp == 0), stop=(jp == HP - 1),
                                        perf_mode=DR,
                                    )
                                nc.scalar.activation(
                                    v_sb[:, tb * 4 + mt, :], psvt[:], AF.Copy,
                                    scale=V_DQ)

                        for tb in range(NCORES):
                            qkv_tb(tb)

                    with (
                        tc.tile_pool(name="atsb", bufs=1) as atsb,
                        tc.tile_pool(name="cp", bufs=4) as cp,
                        tc.tile_pool(name="pss", bufs=2, space="PSUM") as pss_p,
                        tc.tile_pool(name="pso", bufs=2, space="PSUM") as pso_p,
                        tc.tile_pool(name="psdn", bufs=2,
                                     space="PSUM") as psdn_p,
                    ):
                        at_sb = [atsb.tile([128, B * SB, TPC], F8,
                                           name=f"at{m}", tag=f"at{m}")
                                 for m in range(NH)]

                        def attention(m, b):
                            msl = slice(64 * m, 64 * m + 64)
                            for qb in range(SB):
                                qsl = slice(b * S + qb * TPC,
                                            b * S + (qb + 1) * TPC)
                                pso = pso_p.tile([128, TPC], F32, tag="o")
                                psden = psdn_p.tile([1, TPC], F32, tag="dn")
                                for kg in range(KCN // 2):
                                    pss = pss_p.tile([128, 2 * TPC], F32,
                                                     tag="s")
                                    for h_ in range(2):
                                        kc = 2 * kg + h_
                                        ko = b * S + kc * 128
                                        nc.tensor.matmul(
                                            pss[:, h_ * TPC:(h_ + 1) * TPC],
                                            krot[msl, :, ko:ko + 128],
                                            qrot[msl, :, qsl],
                                            start=True, stop=True,
                                            perf_mode=DR,
                                        )
                                    pt = cp.tile([128, 2, TPC], F8, tag="pt")
                                    nc.scalar.activation(
                                        pt[:], pss[:], AF.Exp,
                                        scale=EXP_SCALE, bias=expb_b[:])
                                    ti = b * KCN + 2 * kg
                                    nc.tensor.matmul(
                                        pso[:],
                                        v_sb[:, ti:ti + 2,
                                             m * 128:(m + 1) * 128],
                                        pt[:],
                                        start=(kg == 0),
                                        stop=(kg == KCN // 2 - 1),
                                        perf_mode=DR,
                                    )
                                    nc.tensor.matmul(
                                        psden[:], ones8[:, :, 0:1], pt[:],
                                        start=(kg == 0),
                                        stop=(kg == KCN // 2 - 1),
                                        perf_mode=DR,
                                    )
                                rec = cp.tile([1, TPC], F32, tag="rec")
                                nc.vector.reciprocal(rec[:], psden[:])
                                rb = cp.tile([128, TPC], F32, tag="rbs")
                                nc.gpsimd.partition_broadcast(rb[:], rec[:])
                                nc.vector.tensor_mul(
                                    at_sb[m][:, b * SB + qb, :], pso[:], rb[:])

                        for m in range(NH):
                            for b in range(B):
                                attention(m, b)
                                if b == B - 1:
                                    for r in range(NCORES):
                                        nc.sync.dma_start(
                                            a2a_in_m[m][r * 128:
                                                        (r + 1) * 128, :],
                                            at_sb[m][:, r, :])
                                    if single_core:
                                        a2a_mid = dram.tile(
                                            [NCORES * 128, TPC], F8,
                                            name=f"a2am{m}")
                                        nc.sync.dma_start(a2a_mid[:, :],
                                                          a2a_in_m[m][:, :])
                                        nc.sync.dma_start(a2a_out_m[m][:, :],
                                                          a2a_mid[:, :])
                                    else:
                                        nc.gpsimd.collective_compute(
                                            "AllToAll", mybir.AluOpType.bypass,
                                            replica_groups=rg,
                                            ins=[a2a_in_m[m].opt()],
                                            outs=[a2a_out_m[m].opt()],
                                        )

                    # ---- Stage D: out_proj + residual ----
                    O_DQ = 1.0 / (S_V * S_WO)
                    with (
                        tc.tile_pool(name="atp", bufs=1) as atp,
                        tc.tile_pool(name="pso2", bufs=4,
                                     space="PSUM") as pso2_p,
                    ):
                        at2 = atp.tile([128, HC, TPC], F8, tag="at2")
                        for j in range(HC):
                            src = a2a_out_m[j % 2]
                            ro = (j // 2) * 128
                            nc.sync.dma_start(at2[:, j, :], src[ro:ro + 128, :])
                        for mo in range(HC):
                            ps = pso2_p.tile([128, TPC], F32, tag="o2")
                            mosl = slice(mo * 128, (mo + 1) * 128)
                            for jp in range(HP):
                                nc.tensor.matmul(
                                    ps[:],
                                    wo_sb[:, 2 * jp:2 * jp + 2, mosl],
                                    at2[:, 2 * jp:2 * jp + 2, :],
                                    start=(jp == 0), stop=(jp == HP - 1),
                                    perf_mode=DR,
                                )
                            nc.vector.scalar_tensor_tensor(
                                x2_sb[:, mo, :], ps[:], O_DQ,
                                x_sb[:, mo, :], MULT, ADD)
            # x, qkv outputs, early weights freed here.

            # ================= Stage D2: LN2;  Stage E: MLP ==================
            F1_DQ = 1.0 / (S_H2 * S_W1)
            F2_DQ = 1.0 / (S_W2 * S_FF)
            with (
                tc.tile_pool(name="ffp", bufs=1) as ffp,
                tc.tile_pool(name="gelp", bufs=2) as gelp,
                tc.tile_pool(name="wf2p", bufs=2) as wf2p,
                tc.tile_pool(name="outp", bufs=2) as outp,
            ):
                ff_hi = ffp.tile([128, FFC, TPC], F8, tag="ffh")
                ff_lo = ffp.tile([128, FFC, TPC], F8, tag="ffl")

                with (
                    tc.tile_pool(name="wf1p", bufs=1) as wf1p,
                    tc.tile_pool(name="wf1lp", bufs=2) as wf1lp,
                ):
                    wf1h_sb = wf1p.tile([128, HC, 4096], F8, tag="wf1h")
                    nc.sync.dma_start(wf1h_sb[:], wf1_h[:])

                    with tc.tile_pool(name="h2p", bufs=1) as h2p:
                        h2_hi = h2p.tile([128, HC, TPC], F8, tag="h2hi")
                        h2_lo = h2p.tile([128, HC, TPC], F8, tag="h2lo")
                        with (
                            tc.tile_pool(name="lnD", bufs=3) as lnD,
                            tc.tile_pool(name="psstD", bufs=2,
                                         space="PSUM") as psD,
                            tc.tile_pool(name="psbcD", bufs=2,
                                         space="PSUM") as pbD,
                        ):
                            def put_h2(j, t2, lnp):
                                nc.gpsimd.tensor_scalar(
                                    h2_hi[:, j, :], t2[:], g2_sb[:, j:j + 1],
                                    b2_sb[:, j:j + 1], MULT, ADD)
                                h16 = lnp.tile([128, TPC], F16, tag="h16")
                                nc.gpsimd.tensor_scalar(
                                    h16[:], t2[:], g2_sb[:, j:j + 1],
                                    b2_sb[:, j:j + 1], MULT, ADD)
                                nc.vector.scalar_tensor_tensor(
                                    h2_lo[:, j, :], h2_hi[:, j, :], -1.0,
                                    h16[:], MULT, ADD)

                            layernorm(x2_sb, put_h2, lnD, psD, pbD)

                        # ---- fc1 (3 same-scale chains) + gelu + ff split ----
                        with tc.tile_pool(name="psf1", bufs=4,
                                          space="PSUM") as psf1_p:
                            for mo in range(FFC):
                                w1l = wf1lp.tile([128, HC, 128], F8,
                                                 tag="w1l")
                                nc.sync.dma_start(
                                    w1l[:],
                                    wf1_l[:, mo * 2048:(mo + 1) * 2048])
                                ps = psf1_p.tile([128, TPC], F32, tag="f1")
                                mosl = slice(mo * 128, (mo + 1) * 128)
                                for jp in range(HP):
                                    nc.tensor.matmul(
                                        ps[:],
                                        wf1h_sb[:, 2 * jp:2 * jp + 2, mosl],
                                        h2_hi[:, 2 * jp:2 * jp + 2, :],
                                        start=(jp == 0), stop=False,
                                        perf_mode=DR,
                                    )
                                for jp in range(HP):
                                    nc.tensor.matmul(
                                        ps[:],
                                        w1l[:, 2 * jp:2 * jp + 2, :],
                                        h2_hi[:, 2 * jp:2 * jp + 2, :],
                                        start=False, stop=False,
                                        perf_mode=DR,
                                    )
                                for jp in range(HP):
                                    nc.tensor.matmul(
                                        ps[:],
                                        wf1h_sb[:, 2 * jp:2 * jp + 2, mosl],
                                        h2_lo[:, 2 * jp:2 * jp + 2, :],
                                        start=False, stop=(jp == HP - 1),
                                        perf_mode=DR,
                                    )
                                g16 = gelp.tile([128, TPC], F16, tag="g16")
                                nc.scalar.activation(
                                    g16[:], ps[:], AF.Gelu, bias=zero_b[:],
                                    scale=F1_DQ)
                                nc.vector.tensor_scalar_mul(
                                    ff_hi[:, mo, :], g16[:], S_FF)
                                nc.vector.scalar_tensor_tensor(
                                    ff_lo[:, mo, :], ff_hi[:, mo, :], -1.0,
                                    g16[:], MULT, ADD)

                # ---- fc2 (3 same-scale chains) + residual + store ----
                with tc.tile_pool(name="psf2", bufs=4, space="PSUM") as psf2_p:
                    for mo in range(HC):
                        w2h = wf2p.tile([128, FFC, 128], F8, tag="w2h")
                        w2l = wf2p.tile([128, FFC, 128], F8, tag="w2l")
                        nc.sync.dma_start(
                            w2h[:], wf2_h[:, mo * 4096:(mo + 1) * 4096])
                        nc.sync.dma_start(
                            w2l[:], wf2_l[:, mo * 4096:(mo + 1) * 4096])
                        ps = psf2_p.tile([128, TPC], F32, tag="f2")
                        chains = ((w2h, ff_hi), (w2l, ff_hi), (w2h, ff_lo))
                        for ci, (wt, f_) in enumerate(chains):
                            for jp in range(FFC // 2):
                                nc.tensor.matmul(
                                    ps[:],
                                    wt[:, 2 * jp:2 * jp + 2, :],
                                    f_[:, 2 * jp:2 * jp + 2, :],
                                    start=(ci == 0 and jp == 0),
                                    stop=(ci == 2 and jp == FFC // 2 - 1),
                                    perf_mode=DR,
                                )
                        ot = outp.tile([128, TPC], F16, tag="ot")
                        nc.vector.scalar_tensor_tensor(
                            ot[:], ps[:], F2_DQ, x2_sb[:, mo, :], MULT, ADD)
                        nc.sync.dma_start(outT[:, mo * TPC:(mo + 1) * TPC],
                                          ot[:])
    return nc


def _build():
    if "nc" in _CACHE:
        return _CACHE["nc"]
    nc = bacc.Bacc(
        "TRN2", target_bir_lowering=False, debug=False,
        enable_asserts=True, num_devices=NCORES,
    )
    _emit(nc)
    nc.compile()
    _CACHE["nc"] = nc
    return nc


E4MAX = 240.0


def _q8(x, scale):
    import ml_dtypes
    v = np.clip(np.asarray(x, np.float32) * scale, -E4MAX, E4MAX)
    return v.astype(ml_dtypes.float8_e4m3fn)


def _q8_split(x, scale):
    hi = _q8(x, scale)
    res = np.asarray(x, np.float32) - hi.astype(np.float32) / scale
    lo = _q8(res, scale)     # same scale: residual sits in subnormal range
    return hi, lo


def _strips_dr(w, scale, split):
    """w [M, K] -> DoubleRow lhsT layout [128, K/128, M] fp8 (optionally +lo)."""
    M, K = w.shape
    kc = K // 128
    t = np.asarray(w, np.float32).T.reshape(kc, 128, M).transpose(1, 0, 2)
    if split:
        hi, lo = _q8_split(t, scale)
        return hi.reshape(128, -1), lo.reshape(128, -1)
    return _q8(t, scale).reshape(128, -1), None


def prepare_inputs(x, pe, w_qkv, w_out, w_fc1, w_fc2, g1, b1, g2, b2):
    x = np.asarray(x, np.float32)
    pe = np.asarray(pe, np.float32)
    w_qkv = np.asarray(w_qkv, np.float32)

    xf = x.reshape(TOK, HID)
    evens = np.arange(0, D, 2)
    odds = np.arange(1, D, 2)

    ropeC = np.tile(pe[:, 0::2].T, (2, B)).astype(np.float16)   # [128, TOK]
    ropeS = np.tile(pe[:, 1::2].T, (2, B)).astype(np.float16)

    gb1 = [np.ascontiguousarray(
        np.asarray(v, np.float32).reshape(HC, 128).T) * S_H
        for v in (g1, b1)]
    gb2 = [np.ascontiguousarray(
        np.asarray(v, np.float32).reshape(HC, 128).T) * S_H2
        for v in (g2, b2)]

    wo_hi, _ = _strips_dr(np.asarray(w_out, np.float32), S_WO, False)
    wf1_hi, wf1_lo3 = _strips_dr(np.asarray(w_fc1, np.float32), S_W1, True)
    w2h3, w2l3 = _strips_dr(np.asarray(w_fc2, np.float32), S_W2, True)

    # streamed weights: regroup [128, kc, nmo*128] -> [128, nmo, kc, 128]
    # so each output-chunk strip is one contiguous DMA.
    def _regroup(a, kc, nmo):
        a = a.reshape(128, kc, nmo, 128).transpose(0, 2, 1, 3)
        return np.ascontiguousarray(a.reshape(128, -1))
    wf1_lo = _regroup(wf1_lo3, HC, FFC)
    wf2_hi, wf2_lo = _regroup(w2h3, FFC, HC), _regroup(w2l3, FFC, HC)

    in_maps = []
    for c in range(NCORES):
        hms = [NH * c + i for i in range(NH)]
        # strips: q_even(head-stacked), q_odd, k_even, k_odd
        rows = []
        for base in (0, HID):   # q then k
            for par in (evens, odds):
                rows.append(np.concatenate(
                    [w_qkv[base + h * D + par] for h in hms]))
        qk = np.concatenate(rows)                      # [512, HID]
        vrows = np.concatenate(
            [w_qkv[2 * HID + h * D:2 * HID + (h + 1) * D] for h in hms])
        wqk_h, _ = _strips_dr(qk, S_W, False)
        wv_h, _ = _strips_dr(vrows, S_W, False)

        xTc = xf[c * TPC:(c + 1) * TPC].T              # [HID, TPC]
        xpm = xTc.reshape(HC, 128, TPC).transpose(1, 0, 2)
        im = {
            "xT": np.ascontiguousarray(
                xpm.reshape(128, HC * TPC)).astype(np.float16),
            "wqk": wqk_h, "wv": wv_h, "wo": wo_hi,
            "wf1_h": wf1_hi, "wf1_l": wf1_lo,
            "wf2_h": wf2_hi, "wf2_l": wf2_lo,
            "g1": gb1[0], "b1": gb1[1], "g2": gb2[0], "b2": gb2[1],
            "ropeC": ropeC, "ropeS": ropeS,
        }
        in_maps.append(im)
    return in_maps


def run(in_maps, **kwargs):
    nc = _build()
    return bass_utils.run_bass_kernel_spmd(
        nc, in_maps, core_ids=list(range(NCORES)), **kwargs
    )


def kernel(x, pe, w_qkv, w_out, w_fc1, w_fc2, g1, b1, g2, b2):
    in_maps = prepare_inputs(x, pe, w_qkv, w_out, w_fc1, w_fc2, g1, b1, g2, b2)
    res = run(in_maps)
    outs = []
    for c in range(NCORES):
        o = res.results[c]["outT"].astype(np.float32)    # [128, HC*TPC]
        o = o.reshape(128, HC, TPC).transpose(1, 0, 2).reshape(HID, TPC)
        outs.append(o)
    fullT = np.concatenate(outs, axis=1)                 # [HID, TOK]
    return np.ascontiguousarray(fullT.T).reshape(B, S, HID).astype(np.float32)
